# revision 2
# baseline (speedup 1.0000x reference)
"""Trainium2 Bass kernel for the DSFS dense-MLP problem.

Computation (per batch row b):
    h[b,f,:]  = relu(x[b,f,:] @ feat_w[f] + feat_b[f])      f in 0..31, E=H=64
    hf[b,:]   = h[b].reshape(2048)
    res[b,:]  = relu(hf[b] @ trans_w + trans_b) + hf[b]
    weight[b] = relu(res[b] @ tw_w + tw_b).reshape(64, 128)
    bias[b]   = relu(res[b] @ tb_w + tb_b)

Strategy: pure data-parallel over 8 NeuronCores (512 batch rows each).
On-chip activations are kept feature-major ([features, batch]), so that
  out.T = W.T @ in.T  ==  matmul(psum, lhsT=W_block, rhs=inT_block)
with every weight block a natural row-major slice of the DRAM weight, and
the per-feature bias a per-partition scalar fused into the ReLU activation.
The host transposes x into [2048, 512] per core and transposes the two
outputs back (host work is not part of HW exec time).

Matmuls run as float32r (full PE rate; TF32-class rounding), accumulating
in fp32 PSUM.
"""

import os

import numpy as np

import concourse.bacc as bacc
import concourse.mybir as mybir
import concourse.tile as tile
from concourse.bass import ts
from concourse.bass_utils import run_bass_kernel_spmd

F32 = mybir.dt.float32
F32R = mybir.dt.float32r
RELU = mybir.ActivationFunctionType.Relu

N_CORES = 8
B = 4096
BS = B // N_CORES          # 512 batch rows per core
FH = 2048                  # F*H = flattened feature dim
KC = FH // 128             # 16 contraction chunks of 128
D0, D1 = 64, 128
M2 = FH * 4                # 8192 = D0*D1 output features of tw_w

_CACHE = {}


def _build():
    nc = bacc.Bacc("TRN2", target_bir_lowering=False, debug=False,
                   num_devices=N_CORES)

    xT = nc.declare_dram_parameter("xT", [FH, BS], F32R, isOutput=False)
    fw = nc.declare_dram_parameter("fw", [KC, 128, 128], F32R, isOutput=False)
    w1 = nc.declare_dram_parameter("w1", [FH, FH], F32R, isOutput=False)
    w2 = nc.declare_dram_parameter("w2", [FH, M2], F32R, isOutput=False)
    w3 = nc.declare_dram_parameter("w3", [FH, D1], F32R, isOutput=False)
    # biases, packed: cols [0:16]=feat_b, [16:32]=trans_b, [32:96]=tw_b, [96]=tb_b
    bc = nc.declare_dram_parameter("bc", [128, 97], F32, isOutput=False)
    wT = nc.declare_dram_parameter("wT", [M2, BS], F32, isOutput=True)
    bT = nc.declare_dram_parameter("bT", [D1, BS], F32, isOutput=True)

    with tile.TileContext(nc) as tc:
        with (
            tc.tile_pool(name="wp", bufs=2) as wp,        # streamed weight blocks (and xT)
            tc.tile_pool(name="res", bufs=1) as resp,     # resident activations
            tc.tile_pool(name="small", bufs=1) as smallp, # fw / biases / w3
            tc.tile_pool(name="tmp", bufs=2) as tmpp,     # stage-2 relu before residual add
            tc.tile_pool(name="out", bufs=3) as outp,     # output staging
            tc.tile_pool(name="ps", bufs=4, space="PSUM") as pp,
        ):
            bias_sb = smallp.tile([128, 97], F32, tag="bias")
            nc.sync.dma_start(bias_sb[:], bc[:])

            fw_sb = smallp.tile([128, KC, 128], F32R, tag="fw")
            nc.sync.dma_start(fw_sb[:], fw.rearrange("n p m -> p n m"))

            x_sb = wp.tile([128, KC, BS], F32R, tag="w")
            nc.sync.dma_start(x_sb[:], xT.rearrange("(n p) b -> p n b", p=128))

            h_sb = resp.tile([128, KC, BS], F32R, tag="h")
            r_sb = resp.tile([128, KC, BS], F32R, tag="r")

            # stage 1: h = relu(block_diag(feat_w).T @ xT + feat_b)
            for i in range(KC):
                ps = pp.tile([128, BS], F32)
                nc.tensor.matmul(ps[:], fw_sb[:, i, :], x_sb[:, i, :],
                                 start=True, stop=True)
                nc.scalar.activation(h_sb[:, i, :], ps[:], RELU,
                                     bias=bias_sb[:, i:i + 1])

            # stage 2: res = relu(trans_w.T @ h + trans_b) + h
            for mb in range(4):
                w_t = wp.tile([128, KC, 512], F32R, tag="w")
                nc.sync.dma_start(
                    w_t[:],
                    w1[:, mb * 512:(mb + 1) * 512].rearrange(
                        "(n p) m -> p n m", p=128))
                for j in range(4):
                    m = mb * 4 + j
                    ps = pp.tile([128, BS], F32)
                    for k in range(KC):
                        nc.tensor.matmul(ps[:], w_t[:, k, ts(j, 128)],
                                         h_sb[:, k, :],
                                         start=(k == 0), stop=(k == KC - 1))
                    t = tmpp.tile([128, BS], F32R, tag="t")
                    nc.scalar.activation(t[:], ps[:], RELU,
                                         bias=bias_sb[:, 16 + m:17 + m])
                    nc.vector.tensor_add(r_sb[:, m, :], t[:], h_sb[:, m, :])

            # stage 3: weight.T = relu(tw_w.T @ res + tw_b)
            for mb in range(16):
                w_t = wp.tile([128, KC, 512], F32R, tag="w")
                nc.sync.dma_start(
                    w_t[:],
                    w2[:, mb * 512:(mb + 1) * 512].rearrange(
                        "(n p) m -> p n m", p=128))
                for j in range(4):
                    m = mb * 4 + j
                    ps = pp.tile([128, BS], F32)
                    for k in range(KC):
                        nc.tensor.matmul(ps[:], w_t[:, k, ts(j, 128)],
                                         r_sb[:, k, :],
                                         start=(k == 0), stop=(k == KC - 1))
                    o = outp.tile([128, BS], F32, tag="o")
                    nc.scalar.activation(o[:], ps[:], RELU,
                                         bias=bias_sb[:, 32 + m:33 + m])
                    nc.sync.dma_start(wT[m * 128:(m + 1) * 128, :], o[:])

            # stage 4: bias.T = relu(tb_w.T @ res + tb_b)
            w3_sb = smallp.tile([128, KC, D1], F32R, tag="w3")
            nc.sync.dma_start(w3_sb[:], w3.rearrange("(n p) m -> p n m", p=128))
            ps = pp.tile([128, BS], F32)
            for k in range(KC):
                nc.tensor.matmul(ps[:], w3_sb[:, k, :], r_sb[:, k, :],
                                 start=(k == 0), stop=(k == KC - 1))
            o = outp.tile([128, BS], F32, tag="o")
            nc.scalar.activation(o[:], ps[:], RELU,
                                 bias=bias_sb[:, 96:97])
            nc.sync.dma_start(bT[:, :], o[:])

    nc.compile()
    return nc


def kernel(x, feat_w, feat_b, trans_w, trans_b, tw_w, tw_b, tb_w, tb_b):
    x = np.ascontiguousarray(np.asarray(x, dtype=np.float32))
    feat_w = np.asarray(feat_w, dtype=np.float32)

    # block-diagonal pack of the 32 per-feature [64,64] weights, 2 per 128-row tile
    fw_bd = np.zeros((KC, 128, 128), dtype=np.float32)
    for i in range(KC):
        fw_bd[i, :64, :64] = feat_w[2 * i]
        fw_bd[i, 64:, 64:] = feat_w[2 * i + 1]

    bc = np.zeros((128, 97), dtype=np.float32)
    bc[:, 0:16] = np.asarray(feat_b, np.float32).reshape(16, 128).T
    bc[:, 16:32] = np.asarray(trans_b, np.float32).reshape(16, 128).T
    bc[:, 32:96] = np.asarray(tw_b, np.float32).reshape(64, 128).T
    bc[:, 96] = np.asarray(tb_b, np.float32)

    common = {
        "fw": np.ascontiguousarray(fw_bd),
        "w1": np.ascontiguousarray(np.asarray(trans_w, np.float32)),
        "w2": np.ascontiguousarray(np.asarray(tw_w, np.float32)),
        "w3": np.ascontiguousarray(np.asarray(tb_w, np.float32)),
        "bc": bc,
    }
    in_maps = []
    for c in range(N_CORES):
        xc = x[c * BS:(c + 1) * BS].reshape(BS, FH)
        in_maps.append({"xT": np.ascontiguousarray(xc.T), **common})

    if "nc" not in _CACHE:
        _CACHE["nc"] = _build()
    nc = _CACHE["nc"]
    _CACHE["last_in_maps"] = in_maps

    res = run_bass_kernel_spmd(nc, in_maps, core_ids=list(range(N_CORES)))

    weight = np.empty((B, D0, D1), dtype=np.float32)
    bias = np.empty((B, D1), dtype=np.float32)
    for c in range(N_CORES):
        weight[c * BS:(c + 1) * BS] = res.results[c]["wT"].T.reshape(BS, D0, D1)
        bias[c * BS:(c + 1) * BS] = res.results[c]["bT"].T
    return weight, bias


# revision 3
# speedup vs baseline: 1.4898x; 1.4898x over previous
"""Trainium2 Bass kernel for the DSFS dense-MLP problem.

Computation (per batch row b):
    h[b,f,:]  = relu(x[b,f,:] @ feat_w[f] + feat_b[f])      f in 0..31, E=H=64
    hf[b,:]   = h[b].reshape(2048)
    res[b,:]  = relu(hf[b] @ trans_w + trans_b) + hf[b]
    weight[b] = relu(res[b] @ tw_w + tw_b).reshape(64, 128)
    bias[b]   = relu(res[b] @ tb_w + tb_b)

Strategy: pure data-parallel over 8 NeuronCores (512 batch rows each).
On-chip activations are kept feature-major ([features, batch]), so that
  out.T = W.T @ in.T  ==  matmul(psum, lhsT=W_block, rhs=inT_block)
with every weight block a natural row-major slice of the DRAM weight, and
the per-feature bias a per-partition scalar fused into the ReLU activation.
The host transposes x into [2048, 512] per core and transposes the two
outputs back (host work is not part of HW exec time).

Matmuls run as float32r (fp32 storage), accumulating in fp32 PSUM.
"""

import os

import numpy as np

import concourse.bacc as bacc
import concourse.mybir as mybir
import concourse.tile as tile
from concourse.bass import ts
from concourse.bass_utils import run_bass_kernel_spmd

F32 = mybir.dt.float32
F32R = mybir.dt.float32r
RELU = mybir.ActivationFunctionType.Relu

N_CORES = 8
B = 4096
BS = B // N_CORES          # 512 batch rows per core
FH = 2048                  # F*H = flattened feature dim
KC = FH // 128             # 16 contraction chunks of 128
D0, D1 = 64, 128
M2 = FH * 4                # 8192 = D0*D1 output features of tw_w

_CACHE = {}


def _emit(tc, nc, h):
    xT, fw, w1, w2, w3, bc, wT, bT = h
    with (
        tc.tile_pool(name="wp", bufs=2) as wp,        # streamed weight blocks (and xT)
        tc.tile_pool(name="res", bufs=1) as resp,     # resident activations
        tc.tile_pool(name="small", bufs=1) as smallp, # fw / biases / w3
        tc.tile_pool(name="tmp", bufs=2) as tmpp,     # stage-2 relu before residual add
        tc.tile_pool(name="out", bufs=3) as outp,     # output staging
        tc.tile_pool(name="ps", bufs=4, space="PSUM") as pp,
    ):
        bias_sb = smallp.tile([128, 97], F32, tag="bias")
        nc.sync.dma_start(bias_sb[:], bc[:])

        fw_sb = smallp.tile([128, KC, 128], F32R, tag="fw")
        nc.sync.dma_start(fw_sb[:], fw.rearrange("n p m -> p n m"))

        x_sb = wp.tile([128, KC, BS], F32R, tag="w")
        nc.sync.dma_start(x_sb[:], xT.rearrange("(n p) b -> p n b", p=128))

        h_sb = resp.tile([128, KC, BS], F32R, tag="h")
        r_sb = resp.tile([128, KC, BS], F32R, tag="r")

        # stage 1: h = relu(block_diag(feat_w).T @ xT + feat_b)
        for i in range(KC):
            ps = pp.tile([128, BS], F32)
            nc.tensor.matmul(ps[:], fw_sb[:, i, :], x_sb[:, i, :],
                             start=True, stop=True)
            nc.scalar.activation(h_sb[:, i, :], ps[:], RELU,
                                 bias=bias_sb[:, i:i + 1])

        # stage 2: res = relu(trans_w.T @ h + trans_b) + h
        for mb in range(4):
            w_t = wp.tile([128, KC, 512], F32R, tag="w")
            nc.sync.dma_start(
                w_t[:],
                w1[:, mb * 512:(mb + 1) * 512].rearrange(
                    "(n p) m -> p n m", p=128))
            for j in range(4):
                m = mb * 4 + j
                ps = pp.tile([128, BS], F32)
                for k in range(KC):
                    nc.tensor.matmul(ps[:], w_t[:, k, ts(j, 128)],
                                     h_sb[:, k, :],
                                     start=(k == 0), stop=(k == KC - 1))
                t = tmpp.tile([128, BS], F32R, tag="t")
                nc.scalar.activation(t[:], ps[:], RELU,
                                     bias=bias_sb[:, 16 + m:17 + m])
                nc.vector.tensor_add(r_sb[:, m, :], t[:], h_sb[:, m, :])

        # stage 3: weight.T = relu(tw_w.T @ res + tw_b)
        for mb in range(16):
            w_t = wp.tile([128, KC, 512], F32R, tag="w")
            nc.sync.dma_start(
                w_t[:],
                w2[:, mb * 512:(mb + 1) * 512].rearrange(
                    "(n p) m -> p n m", p=128))
            for j in range(4):
                m = mb * 4 + j
                ps = pp.tile([128, BS], F32)
                for k in range(KC):
                    nc.tensor.matmul(ps[:], w_t[:, k, ts(j, 128)],
                                     r_sb[:, k, :],
                                     start=(k == 0), stop=(k == KC - 1))
                o = outp.tile([128, BS], F32, tag="o")
                nc.scalar.activation(o[:], ps[:], RELU,
                                     bias=bias_sb[:, 32 + m:33 + m])
                nc.sync.dma_start(wT[m * 128:(m + 1) * 128, :], o[:])

        # stage 4: bias.T = relu(tb_w.T @ res + tb_b)
        w3_sb = smallp.tile([128, KC, D1], F32R, tag="w3")
        nc.sync.dma_start(w3_sb[:], w3.rearrange("(n p) m -> p n m", p=128))
        ps = pp.tile([128, BS], F32)
        for k in range(KC):
            nc.tensor.matmul(ps[:], w3_sb[:, k, :], r_sb[:, k, :],
                             start=(k == 0), stop=(k == KC - 1))
        o = outp.tile([128, BS], F32, tag="o")
        nc.scalar.activation(o[:], ps[:], RELU,
                             bias=bias_sb[:, 96:97])
        nc.sync.dma_start(bT[:, :], o[:])


def _build(loop_n=1):
    nc = bacc.Bacc("TRN2", target_bir_lowering=False, debug=False,
                   num_devices=N_CORES)

    xT = nc.declare_dram_parameter("xT", [FH, BS], F32R, isOutput=False)
    fw = nc.declare_dram_parameter("fw", [KC, 128, 128], F32R, isOutput=False)
    w1 = nc.declare_dram_parameter("w1", [FH, FH], F32R, isOutput=False)
    w2 = nc.declare_dram_parameter("w2", [FH, M2], F32R, isOutput=False)
    w3 = nc.declare_dram_parameter("w3", [FH, D1], F32R, isOutput=False)
    # biases, packed: cols [0:16]=feat_b, [16:32]=trans_b, [32:96]=tw_b, [96]=tb_b
    bc = nc.declare_dram_parameter("bc", [128, 97], F32, isOutput=False)
    wT = nc.declare_dram_parameter("wT", [M2, BS], F32, isOutput=True)
    bT = nc.declare_dram_parameter("bT", [D1, BS], F32, isOutput=True)

    handles = (xT, fw, w1, w2, w3, bc, wT, bT)
    with tile.TileContext(nc) as tc:
        if loop_n == 1:
            _emit(tc, nc, handles)
        else:
            with tc.For_i(0, loop_n, 1):
                _emit(tc, nc, handles)

    nc.compile()
    return nc


def make_in_maps(x, feat_w, feat_b, trans_w, trans_b, tw_w, tw_b, tb_w, tb_b):
    x = np.ascontiguousarray(np.asarray(x, dtype=np.float32))
    feat_w = np.asarray(feat_w, dtype=np.float32)

    # block-diagonal pack of the 32 per-feature [64,64] weights, 2 per 128-row tile
    fw_bd = np.zeros((KC, 128, 128), dtype=np.float32)
    for i in range(KC):
        fw_bd[i, :64, :64] = feat_w[2 * i]
        fw_bd[i, 64:, 64:] = feat_w[2 * i + 1]

    bc = np.zeros((128, 97), dtype=np.float32)
    bc[:, 0:16] = np.asarray(feat_b, np.float32).reshape(16, 128).T
    bc[:, 16:32] = np.asarray(trans_b, np.float32).reshape(16, 128).T
    bc[:, 32:96] = np.asarray(tw_b, np.float32).reshape(64, 128).T
    bc[:, 96] = np.asarray(tb_b, np.float32)

    common = {
        "fw": np.ascontiguousarray(fw_bd),
        "w1": np.ascontiguousarray(np.asarray(trans_w, np.float32)),
        "w2": np.ascontiguousarray(np.asarray(tw_w, np.float32)),
        "w3": np.ascontiguousarray(np.asarray(tb_w, np.float32)),
        "bc": bc,
    }
    in_maps = []
    for c in range(N_CORES):
        xc = x[c * BS:(c + 1) * BS].reshape(BS, FH)
        in_maps.append({"xT": np.ascontiguousarray(xc.T), **common})
    return in_maps


def kernel(x, feat_w, feat_b, trans_w, trans_b, tw_w, tw_b, tb_w, tb_b):
    in_maps = make_in_maps(x, feat_w, feat_b, trans_w, trans_b,
                           tw_w, tw_b, tb_w, tb_b)
    if "nc" not in _CACHE:
        _CACHE["nc"] = _build()
    nc = _CACHE["nc"]
    _CACHE["last_in_maps"] = in_maps

    res = run_bass_kernel_spmd(nc, in_maps, core_ids=list(range(N_CORES)))

    weight = np.empty((B, D0, D1), dtype=np.float32)
    bias = np.empty((B, D1), dtype=np.float32)
    for c in range(N_CORES):
        weight[c * BS:(c + 1) * BS] = res.results[c]["wT"].T.reshape(BS, D0, D1)
        bias[c * BS:(c + 1) * BS] = res.results[c]["bT"].T
    return weight, bias
